# revision 1
# baseline (speedup 1.0000x reference)
"""TRN2 Bass kernel: MultiHeadSelfAttention (B=4, S=2048, D=1024, H=16, DK=64).

Sharding: 8 cores = 4 batches x 2 head-groups (8 heads each).
Per core: QK path in float32r (TF32-ish, 1 cyc/row), V/P path bf16,
softmax via reduce_max + ACT exp(bias=-max), P^T via DMA-transpose (xbar),
PV with [V|1]-stationary -> [O^T; denom], 1/denom broadcast via gpsimd
partition_broadcast, normalization fused into the O^T eviction multiply,
output projection from O^T, partial Y out.
Host: pre-mask x (zeroed masked rows -> masked keys get score 0 -> exp
underflows to exact 0 like the reference's -1e6), pre-transpose x,
permute W columns to [head][dk], fold 1/sqrt(DK) into WQ; final
abs((Y0+Y1)*mask) on host after summing the two head-group partials.
"""

import os
import numpy as np

B, S, D, H, DK = 4, 2048, 1024, 16, 64
HG = 2            # head groups (tensor-parallel)
HL = H // HG      # heads per core = 8
DH = HL * DK      # 512 per-core head width
KT = D // 128     # 8 contraction tiles
NQ = S // 128     # 16 q tiles
NKC = S // 128    # 16 key chunks
QB = 4            # q blocks
QBW = S // QB     # 512 q block width

_cache = {}


def _build():
    from concourse import bacc
    import concourse.mybir as mybir
    import concourse.tile as tile
    from concourse.masks import make_identity

    f32 = mybir.dt.float32
    f32r = mybir.dt.float32r
    bf16 = mybir.dt.bfloat16
    Exp = mybir.ActivationFunctionType.Exp
    AX = mybir.AxisListType.X

    nc = bacc.Bacc("TRN2", target_bir_lowering=False, debug=False, num_devices=8)

    xT_d = nc.dram_tensor("xT", [D, S], f32, kind="ExternalInput")
    wq_d = nc.dram_tensor("wq", [D, DH], f32, kind="ExternalInput")
    wk_d = nc.dram_tensor("wk", [D, DH], f32, kind="ExternalInput")
    wv_d = nc.dram_tensor("wv", [D, DH], f32, kind="ExternalInput")
    wo_d = nc.dram_tensor("wo", [DH, D], f32, kind="ExternalInput")
    y_d = nc.dram_tensor("y", [S, D], f32, kind="ExternalOutput")

    with tile.TileContext(nc) as tc:
        with (
            tc.tile_pool(name="persist", bufs=1) as pp,
            tc.tile_pool(name="psA", bufs=int(os.environ.get("PSA", "7")), space="PSUM") as psA,
            tc.tile_pool(name="psC", bufs=1, space="PSUM") as psC,
        ):
            qT = pp.tile([128, KT // 2, S], f32r, tag="qT")   # (512,2048) 4 ptiles
            kT = pp.tile([128, KT // 2, S], f32r, tag="kT")
            # V with a ones column per head: blocks of 66 = [V_h(64) | 1 | pad]
            v_sb = pp.tile([128, NKC, HL, 66], bf16, tag="v")
            nc.gpsimd.memset(v_sb[:, :, :, 64:65], 1.0)
            wor = pp.tile([128, 4, D], f32r, tag="wor")
            nc.gpsimd.dma_start(wor[:], wo_d.rearrange("(t p) n -> p t n", p=128))

            # ---- phase 1: projections ----
            with (
                tc.tile_pool(name="ph1x", bufs=1) as px,
                tc.tile_pool(name="ph1w", bufs=10) as pw,
                tc.tile_pool(name="ph1wv", bufs=1) as pwv,
            ):
                xr = px.tile([128, KT, S], f32r, tag="xr")
                nc.gpsimd.dma_start(
                    xr[:], xT_d.rearrange("(t p) s -> p t s", p=128)
                )
                wvr = pwv.tile([128, KT, DH], f32r, tag="wvr")
                nc.gpsimd.dma_start(
                    wvr[:], wv_d.rearrange("(t p) n -> p t n", p=128)
                )
                for w_d, dst in ((wq_d, qT), (wk_d, kT)):
                    for p in range(4):
                        wchs = []
                        for k in range(KT):
                            wch = pw.tile([128, 128], f32r, tag="wch")
                            nc.gpsimd.dma_start(
                                wch[:],
                                w_d[k * 128:(k + 1) * 128, p * 128:(p + 1) * 128],
                            )
                            wchs.append(wch)
                        for n in range(4):
                            ps = psA.tile([128, 512], f32, tag="mm")
                            for k in range(KT):
                                nc.tensor.matmul(
                                    ps[:],
                                    wchs[k][:],
                                    xr[:, k, n * 512:(n + 1) * 512],
                                    start=(k == 0),
                                    stop=(k == KT - 1),
                                )
                            nc.vector.tensor_copy(
                                dst[:, p, n * 512:(n + 1) * 512], ps[:]
                            )
                for sc in range(NKC):
                    psv = psA.tile([128, 512], f32, tag="mm")
                    for k in range(KT):
                        nc.tensor.matmul(
                            psv[:],
                            xr[:, k, sc * 128:(sc + 1) * 128],
                            wvr[:, k, :],
                            start=(k == 0),
                            stop=(k == KT - 1),
                        )
                    nc.vector.tensor_copy(
                        v_sb[:, sc, :, 0:64],
                        psv[:].rearrange("p (h w) -> p h w", w=64),
                    )

            # ---- phase 2: attention + output projection ----
            with (
                tc.tile_pool(name="ptb", bufs=int(os.environ.get("PTB", "2")), space="SBUF") as ptbp,
                tc.tile_pool(name="pexp", bufs=int(os.environ.get("PEXP", "3"))) as pexp,
                tc.tile_pool(name="stats", bufs=4) as st,
                tc.tile_pool(name="oTp", bufs=2) as oTp,
                tc.tile_pool(name="yp", bufs=3) as yp,
            ):
                for qb in range(QB):
                    oT = oTp.tile([128, 4, QBW], f32r, tag="oT")
                    for hh in range(HL):
                        p, r0 = hh // 2, (hh % 2) * 64
                        ptb = ptbp.tile([128, QBW // 128, NKC, 128], bf16, tag="ptb")
                        for il in range(QBW // 128):
                            i = qb * 4 + il
                            sq = []
                            for n in range(4):
                                t = psA.tile([128, 512], f32, tag="mm")
                                nc.tensor.matmul(
                                    t[:],
                                    qT[r0:r0 + DK, p, i * 128:(i + 1) * 128],
                                    kT[r0:r0 + DK, p, n * 512:(n + 1) * 512],
                                    start=True,
                                    stop=True,
                                )
                                sq.append(t)
                            mx4 = st.tile([128, 4], f32, tag="mx4")
                            for n in range(4):
                                nc.vector.reduce_max(
                                    mx4[:, n:n + 1], sq[n][:], axis=AX
                                )
                            nm = st.tile([128, 1], f32, tag="nm")
                            nc.vector.tensor_reduce(
                                nm[:], mx4[:], axis=AX,
                                op=mybir.AluOpType.max, negate=True,
                            )
                            p_sb = pexp.tile([128, S], bf16, tag="p")
                            for n in range(4):
                                nc.scalar.activation(
                                    p_sb[:, n * 512:(n + 1) * 512],
                                    sq[n][:],
                                    Exp,
                                    bias=nm[:],
                                    scale=1.0,
                                )
                            nc.sync.dma_start(
                                ptb[:, il, :, :],
                                p_sb[:],
                                transpose=True,
                            )
                        # PV with [V_h | 1] stationary -> [O^T ; denom-row]
                        ot_ps = psC.tile([65, QBW], f32, tag="ot")
                        for kc in range(NKC):
                            nc.tensor.matmul(
                                ot_ps[:],
                                v_sb[:, kc, hh, 0:65],
                                ptb[:, :, kc, :],
                                start=(kc == 0),
                                stop=(kc == NKC - 1),
                            )
                        # recip of denom row, broadcast to 64 partitions
                        rrow = st.tile([1, QBW], f32, tag="rrow")
                        nc.vector.reciprocal(rrow[:], ot_ps[64:65, :])
                        rb = st.tile([64, QBW], f32, tag="rb")
                        nc.gpsimd.partition_broadcast(rb[:], rrow[:])
                        nc.vector.tensor_mul(
                            oT[r0:r0 + 64, p, :], ot_ps[0:64, :], rb[:]
                        )
                    for il in range(QBW // 128):
                        i = qb * 4 + il
                        y_sb = yp.tile([128, D], f32, tag="y")
                        for half in range(2):
                            yq = psA.tile([128, 512], f32, tag="mm")
                            for pp_ in range(4):
                                nc.tensor.matmul(
                                    yq[:],
                                    oT[:, pp_, il * 128:(il + 1) * 128],
                                    wor[:, pp_, half * 512:(half + 1) * 512],
                                    start=(pp_ == 0),
                                    stop=(pp_ == 3),
                                )
                            nc.scalar.copy(
                                y_sb[:, half * 512:(half + 1) * 512], yq[:])
                        nc.sync.dma_start(y_d[i * 128:(i + 1) * 128, :], y_sb[:])

    nc.compile()
    return nc


def _prep_inputs(x, mask, WQ, WK, WV, WO):
    xm = (x.astype(np.float32) * mask.astype(np.float32)[:, :, None])
    in_maps = []
    for c in range(8):
        b, g = c // 2, c % 2
        idx = np.array(
            [dk * H + (g * HL + hh) for hh in range(HL) for dk in range(DK)]
        )
        in_maps.append({
            "xT": np.ascontiguousarray(xm[b].T),
            "wq": np.ascontiguousarray(WQ[:, idx] / np.sqrt(DK)).astype(np.float32),
            "wk": np.ascontiguousarray(WK[:, idx]).astype(np.float32),
            "wv": np.ascontiguousarray(WV[:, idx]).astype(np.float32),
            "wo": np.ascontiguousarray(WO[g * DH:(g + 1) * DH, :]).astype(np.float32),
        })
    return in_maps


def kernel(x, mask, WQ, WK, WV, WO, _want_results=False, _trace=False):
    from concourse.bass_utils import run_bass_kernel_spmd

    if "nc" not in _cache:
        _cache["nc"] = _build()
    nc = _cache["nc"]
    in_maps = _prep_inputs(np.asarray(x), np.asarray(mask), np.asarray(WQ),
                           np.asarray(WK), np.asarray(WV), np.asarray(WO))
    res = run_bass_kernel_spmd(nc, in_maps, list(range(8)), trace=_trace)
    ys = [res.results[c]["y"] for c in range(8)]
    mk = np.asarray(mask).astype(np.float32)
    out = np.empty((B, S, D), np.float32)
    for b in range(B):
        out[b] = np.abs((ys[2 * b] + ys[2 * b + 1]) * mk[b][:, None])
    if _want_results:
        return out, res
    return out



# revision 6
# speedup vs baseline: 2.5087x; 2.5087x over previous
"""TRN2 Bass kernel: MultiHeadSelfAttention (B=4, S=2048, D=1024, H=16, DK=64).

Sharding: 8 cores = 4 batches x 2 head-groups (8 heads each).

Key optimizations over the v1 kernel:
- Token compaction: the reference multiplies the output by the padding mask
  and masked keys get softmax weight exactly 0 (exp(-1e6-max) underflows), so
  attention only involves the unmasked tokens. The host gathers those (~1024
  of 2048) and pads to SP (multiple of 128); outputs are scattered back.
- fp16 operands everywhere (10-bit mantissa ~= TF32): 1 cyc/row matmuls at
  any width, half the SBUF/DMA of f32.
- P-stationary PV: stationary P^T chunk [keys x q], moving [V_h | 1] -> O in
  [q x dh] layout with the softmax denominator as column 64. Normalization is
  a per-partition tensor_scalar_mul fused into the PSUM eviction (gpsimd).
- One full-row max (DVE) + one full-row exp (Act) per (head, q-tile), reading
  a multi-bank PSUM tile.
- Software-pipelined phase 2 with a lag between the QK/softmax stream and the
  PV/out-projection stream.
"""

import numpy as np

B, S, D, H, DK = 4, 2048, 1024, 16, 64
HG = 2            # head groups (tensor-parallel)
HL = H // HG      # heads per core = 8
DH = HL * DK      # 512 per-core head width
KT = D // 128     # 8 contraction tiles
SP_DEFAULT = 1152

_cache = {}


def _build(SP):
    from concourse import bacc
    import concourse.mybir as mybir
    import concourse.tile as tile

    f32 = mybir.dt.float32
    f16 = mybir.dt.float16
    Exp = mybir.ActivationFunctionType.Exp
    AX = mybir.AxisListType.X
    NT = SP // 128

    nc = bacc.Bacc("TRN2", target_bir_lowering=False, debug=False, num_devices=8)

    xT_d = nc.dram_tensor("xT", [D, SP], f16, kind="ExternalInput")
    wq_d = nc.dram_tensor("wq", [D, DH], f16, kind="ExternalInput")
    wk_d = nc.dram_tensor("wk", [D, DH], f16, kind="ExternalInput")
    wv_d = nc.dram_tensor("wv", [D, DH], f16, kind="ExternalInput")
    wo_d = nc.dram_tensor("wo", [DH, D], f16, kind="ExternalInput")
    y_d = nc.dram_tensor("y", [SP, D], f16, kind="ExternalOutput")

    with tile.TileContext(nc) as tc:
        with (
            tc.tile_pool(name="persist", bufs=1) as pp,
            tc.tile_pool(name="psS", bufs=2, space="PSUM") as psS,
            tc.tile_pool(name="psO", bufs=1, space="PSUM") as psO,
            tc.tile_pool(name="psY", bufs=1, space="PSUM") as psY,
        ):
            qT = pp.tile([128, 4, SP], f16, tag="qT")
            kT = pp.tile([128, 4, SP], f16, tag="kT")
            # V with a ones column per head: blocks of 66 = [V_h(64) | 1 | pad]
            v2 = pp.tile([128, NT, HL, 66], f16, tag="v2")
            nc.gpsimd.memset(v2[:, :, :, 64:65], 1.0)
            wor = pp.tile([128, 4, D], f16, tag="wor")
            nc.gpsimd.dma_start(wor[:], wo_d.rearrange("(c p) n -> p c n", p=128))

            # ---- phase 1: projections ----
            with (
                tc.tile_pool(name="ph1x", bufs=1) as px,
                tc.tile_pool(name="ph1w", bufs=10) as pw,
                tc.tile_pool(name="ph1wv", bufs=1) as pwv,
            ):
                xr = px.tile([128, KT, SP], f16, tag="xr")
                nc.gpsimd.dma_start(xr[:], xT_d.rearrange("(t p) s -> p t s", p=128))
                wvr = pwv.tile([128, KT, DH], f16, tag="wvr")
                nc.gpsimd.dma_start(wvr[:], wv_d.rearrange("(t p) n -> p t n", p=128))

                # column chunks of a row of SP scores / tokens (PSUM banks)
                chunks = []
                c0 = 0
                while c0 < SP:
                    c1 = min(c0 + 512, SP)
                    chunks.append((c0, c1))
                    c0 = c1

                for wi, (w_d, dst) in enumerate(((wk_d, kT), (wq_d, qT))):
                    for p in range(4):
                        wchs = []
                        for k in range(KT):
                            wch = pw.tile([128, 128], f16, tag="wch")
                            nc.gpsimd.dma_start(
                                wch[:],
                                w_d[k * 128:(k + 1) * 128, p * 128:(p + 1) * 128],
                            )
                            wchs.append(wch)
                        ps = psS.tile([128, 3, 512], f32, tag="mmS")
                        for (c0, c1) in chunks:
                            bank, off = c0 // 512, c0 % 512
                            for k in range(KT):
                                nc.tensor.matmul(
                                    ps[:, bank, off:off + (c1 - c0)],
                                    wchs[k][:],
                                    xr[:, k, c0:c1],
                                    start=(k == 0),
                                    stop=(k == KT - 1),
                                )
                        sflat = ps[:].rearrange("p a b -> p (a b)")[:, 0:SP]
                        if (wi * 4 + p) % 2 == 0:
                            nc.vector.tensor_copy(dst[:, p, :], sflat)
                        else:
                            nc.scalar.copy(dst[:, p, :], sflat)
                for sc in range(NT):
                    if sc % 2 == 0:
                        psv = psO.tile([128, 512], f32, tag="mmO")
                    else:
                        psv = psY.tile([128, 512], f32, tag="mmY")
                    for k in range(KT):
                        nc.tensor.matmul(
                            psv[:],
                            xr[:, k, sc * 128:(sc + 1) * 128],
                            wvr[:, k, :],
                            start=(k == 0),
                            stop=(k == KT - 1),
                        )
                    nc.gpsimd.tensor_copy(
                        v2[:, sc, :, 0:64],
                        psv[:].rearrange("p (h w) -> p h w", w=64),
                    )

            # ---- phase 2: attention + output projection (software pipelined)
            with (
                tc.tile_pool(name="pexp", bufs=3) as pexp,
                tc.tile_pool(name="ptbp", bufs=2) as ptbp,
                tc.tile_pool(name="stats", bufs=6) as st,
                tc.tile_pool(name="osbp", bufs=2) as osbp,
                tc.tile_pool(name="oTp", bufs=2) as oTp,
                tc.tile_pool(name="yp", bufs=2) as yp,
            ):
                sched = [(i, h) for i in range(NT) for h in range(HL)]
                LAG = 2
                state = {}

                def issue_qk(i, h):
                    p, r0 = h // 2, (h % 2) * 64
                    ps = psS.tile([128, 3, 512], f32, tag="mmS")
                    for (c0, c1) in chunks:
                        bank, off = c0 // 512, c0 % 512
                        nc.tensor.matmul(
                            ps[:, bank, off:off + (c1 - c0)],
                            qT[r0:r0 + DK, p, i * 128:(i + 1) * 128],
                            kT[r0:r0 + DK, p, c0:c1],
                            start=True,
                            stop=True,
                        )
                    sflat = ps[:].rearrange("p a b -> p (a b)")[:, 0:SP]
                    nm = st.tile([128, 1], f32, tag="nm")
                    nc.vector.tensor_reduce(
                        nm[:], sflat, axis=AX, op=mybir.AluOpType.max, negate=True,
                    )
                    p_sb = pexp.tile([128, SP], f16, tag="p")
                    nc.scalar.activation(p_sb[:], sflat, Exp, bias=nm[:], scale=1.0)
                    ptb = ptbp.tile([128, NT, 128], f16, tag="ptb")
                    nc.sync.dma_start(ptb[:], p_sb[:], transpose=True)
                    state[(i, h)] = ptb

                def issue_pv(i, h):
                    ptb = state.pop((i, h))
                    if h == 0:
                        osb_t = osbp.tile([128, HL, 64], f16, tag="osb")
                        state[("osb", i)] = osb_t
                    o_sb = state[("osb", i)]
                    o_ps = psO.tile([128, 512], f32, tag="mmO")
                    for kc in range(NT):
                        nc.tensor.matmul(
                            o_ps[:, 0:65],
                            ptb[:, kc, :],
                            v2[:, kc, h, 0:65],
                            start=(kc == 0),
                            stop=(kc == NT - 1),
                        )
                    rq = st.tile([128, 1], f32, tag="rq")
                    nc.vector.reciprocal(rq[:], o_ps[:, 64:65])
                    nc.gpsimd.tensor_scalar_mul(o_sb[:, h, :], o_ps[:, 0:64], rq[:])

                def issue_oproj(i):
                    o_sb = state.pop(("osb", i))
                    oT = oTp.tile([128, 4, 128], f16, tag="oT")
                    nc.sync.dma_start(
                        oT[:], o_sb[:].rearrange("p a b -> p (a b)"), transpose=True)
                    y_sb = yp.tile([128, D], f16, tag="y")
                    for half in range(2):
                        yq = psY.tile([128, 512], f32, tag="mmY")
                        for c in range(4):
                            nc.tensor.matmul(
                                yq[:],
                                oT[:, c, :],
                                wor[:, c, half * 512:(half + 1) * 512],
                                start=(c == 0),
                                stop=(c == 3),
                            )
                        nc.scalar.copy(y_sb[:, half * 512:(half + 1) * 512], yq[:])
                    nc.sync.dma_start(y_d[i * 128:(i + 1) * 128, :], y_sb[:])

                for idx in range(len(sched) + LAG):
                    if idx < len(sched):
                        issue_qk(*sched[idx])
                    j = idx - LAG
                    if j >= 0:
                        i2, h2 = sched[j]
                        issue_pv(i2, h2)
                        if h2 == HL - 1:
                            issue_oproj(i2)

    nc.compile()
    return nc


def _prep_inputs(x, mask, WQ, WK, WV, WO, SP):
    idxs = [np.nonzero(mask[b])[0] for b in range(B)]
    in_maps = []
    for c in range(8):
        b, g = c // 2, c % 2
        idx = idxs[b]
        perm = np.array(
            [dk * H + (g * HL + hh) for hh in range(HL) for dk in range(DK)]
        )
        xT = np.zeros((D, SP), np.float16)
        xT[:, :len(idx)] = x[b][idx].T
        in_maps.append({
            "xT": xT,
            "wq": np.ascontiguousarray(WQ[:, perm] / np.sqrt(DK)).astype(np.float16),
            "wk": np.ascontiguousarray(WK[:, perm]).astype(np.float16),
            "wv": np.ascontiguousarray(WV[:, perm]).astype(np.float16),
            "wo": np.ascontiguousarray(WO[g * DH:(g + 1) * DH, :]).astype(np.float16),
        })
    return in_maps, idxs


def kernel(x, mask, WQ, WK, WV, WO, _want_results=False, _trace=False):
    from concourse.bass_utils import run_bass_kernel_spmd

    x = np.asarray(x)
    mask = np.asarray(mask)
    nb_max = int(mask.sum(axis=1).max())
    SP = max(SP_DEFAULT, -(-nb_max // 128) * 128)
    if ("nc", SP) not in _cache:
        _cache[("nc", SP)] = _build(SP)
    nc = _cache[("nc", SP)]
    _cache["nc"] = nc  # convenience alias for external tooling
    in_maps, idxs = _prep_inputs(x, mask, np.asarray(WQ, np.float32),
                                 np.asarray(WK, np.float32),
                                 np.asarray(WV, np.float32),
                                 np.asarray(WO, np.float32), SP)
    res = run_bass_kernel_spmd(nc, in_maps, list(range(8)), trace=_trace)
    out = np.zeros((B, S, D), np.float32)
    for b in range(B):
        idx = idxs[b]
        yb = (res.results[2 * b]["y"].astype(np.float32)
              + res.results[2 * b + 1]["y"].astype(np.float32))
        out[b][idx] = np.abs(yb[:len(idx)])
    if _want_results:
        return out, res
    return out


# revision 9
# speedup vs baseline: 2.7547x; 1.0981x over previous
"""TRN2 Bass kernel: MultiHeadSelfAttention (B=4, S=2048, D=1024, H=16, DK=64).

Sharding: 8 cores = 4 batches x 2 head-groups (8 heads each).

Key optimizations over the v1 kernel:
- Token compaction: the reference multiplies the output by the padding mask
  and masked keys get softmax weight exactly 0 (exp(-1e6-max) underflows), so
  attention only involves the unmasked tokens. The host gathers those (~1024
  of 2048) and pads to SP (multiple of 128); outputs are scattered back.
- fp16 operands everywhere (10-bit mantissa ~= TF32): 1 cyc/row matmuls at
  any width, half the SBUF/DMA of f32.
- P-stationary PV: stationary P^T chunk [keys x q], moving [V_h | 1] -> O in
  [q x dh] layout with the softmax denominator as column 64. Normalization is
  a per-partition tensor_scalar_mul fused into the PSUM eviction (gpsimd).
- One full-row max (DVE) + one full-row exp (Act) per (head, q-tile), reading
  a multi-bank PSUM tile.
- Software-pipelined phase 2 with a lag between the QK/softmax stream and the
  PV/out-projection stream.
"""

import numpy as np

B, S, D, H, DK = 4, 2048, 1024, 16, 64
HG = 2            # head groups (tensor-parallel)
HL = H // HG      # heads per core = 8
DH = HL * DK      # 512 per-core head width
KT = D // 128     # 8 contraction tiles
SP_DEFAULT = 1152

_cache = {}


def _build(SP):
    from concourse import bacc
    import concourse.mybir as mybir
    import concourse.tile as tile

    f32 = mybir.dt.float32
    f16 = mybir.dt.float16
    Exp = mybir.ActivationFunctionType.Exp
    AX = mybir.AxisListType.X
    NT = SP // 128

    nc = bacc.Bacc("TRN2", target_bir_lowering=False, debug=False, num_devices=8)

    xT_d = nc.dram_tensor("xT", [D, SP], f16, kind="ExternalInput")
    wq_d = nc.dram_tensor("wq", [D, DH], f16, kind="ExternalInput")
    wk_d = nc.dram_tensor("wk", [D, DH], f16, kind="ExternalInput")
    wv_d = nc.dram_tensor("wv", [D, DH], f16, kind="ExternalInput")
    wo_d = nc.dram_tensor("wo", [DH, D], f16, kind="ExternalInput")
    y_d = nc.dram_tensor("y", [SP, D], f16, kind="ExternalOutput")

    with tile.TileContext(nc) as tc:
        with (
            tc.tile_pool(name="persist", bufs=1) as pp,
            tc.tile_pool(name="psS", bufs=2, space="PSUM") as psS,
            tc.tile_pool(name="psO", bufs=1, space="PSUM") as psO,
            tc.tile_pool(name="psY", bufs=1, space="PSUM") as psY,
        ):
            qT = pp.tile([128, 4, SP], f16, tag="qT")
            kT = pp.tile([128, 4, SP], f16, tag="kT")
            # V with a ones column per head: blocks of 66 = [V_h(64) | 1 | pad]
            v2 = pp.tile([128, NT, HL, 66], f16, tag="v2")
            nc.gpsimd.memset(v2[:, :, :, 64:65], 1.0)
            wor = pp.tile([128, 4, D], f16, tag="wor")
            nc.gpsimd.dma_start(wor[:], wo_d.rearrange("(c p) n -> p c n", p=128))

            # ---- phase 1: projections ----
            xr = pp.tile([128, KT, SP], f16, tag="xr")
            nc.gpsimd.dma_start(xr[:], xT_d.rearrange("(t p) s -> p t s", p=128))
            wvr = pp.tile([128, KT, DH], f16, tag="wvr")
            nc.gpsimd.dma_start(wvr[:], wv_d.rearrange("(t p) n -> p t n", p=128))
            wkr = pp.tile([128, KT, DH], f16, tag="wkr")
            nc.gpsimd.dma_start(wkr[:], wk_d.rearrange("(t p) n -> p t n", p=128))
            wqr = pp.tile([128, KT, DH], f16, tag="wqr")
            nc.gpsimd.dma_start(wqr[:], wq_d.rearrange("(t p) n -> p t n", p=128))

            # column chunks of a row of SP scores / tokens (PSUM banks)
            chunks = []
            c0 = 0
            while c0 < SP:
                c1 = min(c0 + 512, SP)
                chunks.append((c0, c1))
                c0 = c1

            for wi, (wr, dst) in enumerate(((wkr, kT), (wqr, qT))):
                for p in range(4):
                    ps = psS.tile([128, 3, 512], f32, tag="mmS")
                    for (c0, c1) in chunks:
                        bank, off = c0 // 512, c0 % 512
                        for k in range(KT):
                            nc.tensor.matmul(
                                ps[:, bank, off:off + (c1 - c0)],
                                wr[:, k, p * 128:(p + 1) * 128],
                                xr[:, k, c0:c1],
                                start=(k == 0),
                                stop=(k == KT - 1),
                            )
                    sflat = ps[:].rearrange("p a b -> p (a b)")[:, 0:SP]
                    if (wi * 4 + p) % 2 == 0:
                        nc.vector.tensor_copy(dst[:, p, :], sflat)
                    else:
                        nc.scalar.copy(dst[:, p, :], sflat)
            for sc in range(NT):
                if sc % 2 == 0:
                    psv = psO.tile([128, 512], f32, tag="mmO")
                else:
                    psv = psY.tile([128, 512], f32, tag="mmY")
                for k in range(KT):
                    nc.tensor.matmul(
                        psv[:],
                        xr[:, k, sc * 128:(sc + 1) * 128],
                        wvr[:, k, :],
                        start=(k == 0),
                        stop=(k == KT - 1),
                    )
                nc.gpsimd.tensor_copy(
                    v2[:, sc, :, 0:64],
                    psv[:].rearrange("p (h w) -> p h w", w=64),
                )

            # ---- phase 2: attention + output projection (software pipelined)
            with (
                tc.tile_pool(name="pexp", bufs=3) as pexp,
                tc.tile_pool(name="ptbp", bufs=2) as ptbp,
                tc.tile_pool(name="stats", bufs=6) as st,
                tc.tile_pool(name="osbp", bufs=2) as osbp,
                tc.tile_pool(name="oTp", bufs=2) as oTp,
                tc.tile_pool(name="yp", bufs=2) as yp,
            ):
                sched = [(i, h) for i in range(NT) for h in range(HL)]
                LAG = 3     # QK/softmax stream leads the PV stream
                OLAG = 2    # out-projection lags the last PV of its q-tile
                state = {}

                def issue_qk(i, h):
                    p, r0 = h // 2, (h % 2) * 64
                    ps = psS.tile([128, 3, 512], f32, tag="mmS")
                    for (c0, c1) in chunks:
                        bank, off = c0 // 512, c0 % 512
                        nc.tensor.matmul(
                            ps[:, bank, off:off + (c1 - c0)],
                            qT[r0:r0 + DK, p, i * 128:(i + 1) * 128],
                            kT[r0:r0 + DK, p, c0:c1],
                            start=True,
                            stop=True,
                        )
                    sflat = ps[:].rearrange("p a b -> p (a b)")[:, 0:SP]
                    nm = st.tile([128, 1], f32, tag="nm")
                    nc.vector.tensor_reduce(
                        nm[:], sflat, axis=AX, op=mybir.AluOpType.max, negate=True,
                    )
                    p_sb = pexp.tile([128, SP], f16, tag="p")
                    nc.scalar.activation(p_sb[:], sflat, Exp, bias=nm[:], scale=1.0)
                    ptb = ptbp.tile([128, NT, 128], f16, tag="ptb")
                    nc.sync.dma_start(ptb[:], p_sb[:], transpose=True)
                    state[(i, h)] = ptb

                def issue_pv(i, h):
                    ptb = state.pop((i, h))
                    if h == 0:
                        osb_t = osbp.tile([128, HL, 64], f16, tag="osb")
                        state[("osb", i)] = osb_t
                    o_sb = state[("osb", i)]
                    o_ps = psO.tile([128, 512], f32, tag="mmO")
                    for kc in range(NT):
                        nc.tensor.matmul(
                            o_ps[:, 0:65],
                            ptb[:, kc, :],
                            v2[:, kc, h, 0:65],
                            start=(kc == 0),
                            stop=(kc == NT - 1),
                        )
                    rq = st.tile([128, 1], f32, tag="rq")
                    nc.vector.reciprocal(rq[:], o_ps[:, 64:65])
                    nc.gpsimd.tensor_scalar_mul(o_sb[:, h, :], o_ps[:, 0:64], rq[:])

                def issue_oproj(i):
                    o_sb = state.pop(("osb", i))
                    oT = oTp.tile([128, 4, 128], f16, tag="oT")
                    nc.sync.dma_start(
                        oT[:], o_sb[:].rearrange("p a b -> p (a b)"), transpose=True)
                    y_sb = yp.tile([128, D], f16, tag="y")
                    for half in range(2):
                        yq = psY.tile([128, 512], f32, tag="mmY")
                        for c in range(4):
                            nc.tensor.matmul(
                                yq[:],
                                oT[:, c, :],
                                wor[:, c, half * 512:(half + 1) * 512],
                                start=(c == 0),
                                stop=(c == 3),
                            )
                        nc.scalar.copy(y_sb[:, half * 512:(half + 1) * 512], yq[:])
                    nc.sync.dma_start(y_d[i * 128:(i + 1) * 128, :], y_sb[:])

                n = len(sched)
                for idx in range(n + LAG + OLAG):
                    if idx < n:
                        issue_qk(*sched[idx])
                    j = idx - LAG
                    if 0 <= j < n:
                        issue_pv(*sched[j])
                    k2 = idx - LAG - OLAG
                    if 0 <= k2 < n and sched[k2][1] == HL - 1:
                        issue_oproj(sched[k2][0])

    nc.compile()
    return nc


def _prep_inputs(x, mask, WQ, WK, WV, WO, SP):
    idxs = [np.nonzero(mask[b])[0] for b in range(B)]
    in_maps = []
    for c in range(8):
        b, g = c // 2, c % 2
        idx = idxs[b]
        perm = np.array(
            [dk * H + (g * HL + hh) for hh in range(HL) for dk in range(DK)]
        )
        xT = np.zeros((D, SP), np.float16)
        xT[:, :len(idx)] = x[b][idx].T
        in_maps.append({
            "xT": xT,
            "wq": np.ascontiguousarray(WQ[:, perm] / np.sqrt(DK)).astype(np.float16),
            "wk": np.ascontiguousarray(WK[:, perm]).astype(np.float16),
            "wv": np.ascontiguousarray(WV[:, perm]).astype(np.float16),
            "wo": np.ascontiguousarray(WO[g * DH:(g + 1) * DH, :]).astype(np.float16),
        })
    return in_maps, idxs


def kernel(x, mask, WQ, WK, WV, WO, _want_results=False, _trace=False):
    from concourse.bass_utils import run_bass_kernel_spmd

    x = np.asarray(x)
    mask = np.asarray(mask)
    nb_max = int(mask.sum(axis=1).max())
    SP = max(SP_DEFAULT, -(-nb_max // 128) * 128)
    if ("nc", SP) not in _cache:
        _cache[("nc", SP)] = _build(SP)
    nc = _cache[("nc", SP)]
    _cache["nc"] = nc  # convenience alias for external tooling
    in_maps, idxs = _prep_inputs(x, mask, np.asarray(WQ, np.float32),
                                 np.asarray(WK, np.float32),
                                 np.asarray(WV, np.float32),
                                 np.asarray(WO, np.float32), SP)
    res = run_bass_kernel_spmd(nc, in_maps, list(range(8)), trace=_trace)
    out = np.zeros((B, S, D), np.float32)
    for b in range(B):
        idx = idxs[b]
        yb = (res.results[2 * b]["y"].astype(np.float32)
              + res.results[2 * b + 1]["y"].astype(np.float32))
        out[b][idx] = np.abs(yb[:len(idx)])
    if _want_results:
        return out, res
    return out
